# revision 2
# baseline (speedup 1.0000x reference)
"""CapsNet-EM forward kernel v2 for 8 Trainium2 NeuronCores.

Same degenerate-EM closed form as the baseline kernel (convs -> votes;
ao = sigmoid(3*(beta_a - S * sum_k(beta_v[k] - 0.5*log(30752*v_k^2))))),
restructured for TRN2:

- fp8(e4m3) data path; votes pre-scaled by ALPHA=128 (folded into conv
  weights, 2*ln(alpha) absorbed into the c0 constants) so fp8 holds the
  tiny vote magnitudes.
- Primary-caps scramble staged to DRAM in fp8, read back through the DMA
  xbar as bf16 ATOM PAIRS; L1 votes matmuls contract the two fp8 channels
  of each pair with DoubleRow (ktile = intra-pair fp8 offset).
- L2/L3 votes use DoubleRow with ktile = spatial tap pairs (free-dim
  shifts of the same SBUF tile; the odd 9th tap rides a zero-weight
  ktile with stride 0).
- pc-conv bias (position-scrambled by the torch .view) enters L1 votes as
  a host-computed per-(out-ch, a, b) field added into PSUM via a small
  identity-rhs matmul.
- log(v^2) from the bf16 bit pattern of a Square-activation evacuation of
  the votes PSUM (f32, so tiny votes never quantize to zero), bitcast
  int16 -> fp16 (gpsimd) -> ks matmul.
"""
import sys
import numpy as np

for _p in ("/opt/trn_rl_repo",):
    if _p not in sys.path:
        sys.path.insert(0, _p)

import ml_dtypes

BF = ml_dtypes.bfloat16
F16 = np.float16
FP8 = ml_dtypes.float8_e4m3

# ---------------- model dims ----------------
NCORES = 8
BATCH = 256
IMG = BATCH // NCORES          # 32 images per core
G0, G1, G2 = 14, 6, 4
P0 = G0 * G0                   # 196
N0 = IMG * P0                  # 6272
N1 = IMG * G1 * G1             # 1152
N2 = IMG * G2 * G2             # 512
A1 = 64.0
A2 = 2.0
A3 = 0.25
LAM = float(np.log(2.0) / 256.0)
LOGC = float(np.log(32.0 * 31.0 * 31.0))
C0CONST = float(-8.0 * LOGC + 8.0 * np.log(2.0) * (127.0 - 0.043))

OFF3 = [(d, e) for d in range(3) for e in range(3)]
# L2 tap pairs in flat order j=3d+e: ktile stride = col shift between pair
L2PAIRS = [(0, 1, 1), (2, 3, 4), (4, 5, 1), (6, 7, 1), (8, None, 0)]
L3PAIRS = [(2 * i, 2 * i + 1, 1) for i in range(8)]

# weight blob column maps (host + device share these)
WQC = {"w1": 0, "pcw": 32, "l1w": 576, "l2w": 5184, "l3w": 10304}
QW_Z0 = 13376
QW_TOT = 13504
WBC = {"s1w": 0, "s2w": 288, "s3w": 576, "b1f": 736}
WB_TOT = 880
# img chunking for the 32-wide routing phase of L1
L1CH = [(0, 11), (11, 22), (22, 32)]


def _blkdiag(tw, scale, taps_w):
    """(4, taps, 128, opt) block-diag expansion of grouped conv weights."""
    oc_tot = tw.shape[0]
    opt = oc_tot // 4
    ob = opt // 4
    taps = tw.shape[2] * tw.shape[3]
    out = np.zeros((4, taps, 128, opt), np.float32)
    for t in range(4):
        for oi in range(taps):
            d, e = divmod(oi, taps_w)
            for blk in range(4):
                oc0 = opt * t + ob * blk
                out[t, oi, 32 * blk:32 * blk + 32, ob * blk:ob * blk + ob] = \
                    scale * tw[oc0:oc0 + ob, :, d, e].T
    return out


# ---------------- host-side weight preprocessing ----------------
def prep_weights(inp):
    f32 = np.float32
    o = {}

    wq = np.zeros((128, QW_TOT), f32)

    conv1_w = np.asarray(inp["conv1_w"], f32)      # (32,1,5,5)
    for q in range(4):
        for a in range(5):
            for b in range(5):
                wq[32 * q + 5 * a + b, WQC["w1"]:WQC["w1"] + 32] = \
                    conv1_w[:, 0, a, b]

    pc_w = np.asarray(inp["pc_w"], f32)[:, :, 0, 0]    # (544,32)
    for q in range(4):
        for mt in range(4):
            wq[32 * q:32 * q + 32,
               WQC["pcw"] + 128 * mt:WQC["pcw"] + 128 * (mt + 1)] = \
                pc_w[32 + 128 * mt:32 + 128 * (mt + 1), :].T
        wq[32 * q:32 * q + 32, WQC["pcw"] + 512:WQC["pcw"] + 544] = \
            pc_w[:32, :].T

    l1 = _blkdiag(np.asarray(inp["t1_w"], f32), A1, 3)       # (4,9,128,128)
    for t in range(4):
        r0 = 64 * (t % 2)
        c0 = WQC["l1w"] + 2304 * (t // 2)
        for oi in range(9):
            for i in range(2):
                for k in range(64):
                    wq[r0 + k, c0 + 256 * oi + 128 * i:
                       c0 + 256 * oi + 128 * (i + 1)] = l1[t, oi, 2 * k + i]

    l2 = _blkdiag(np.asarray(inp["t2_w"], f32), 32.0 * A2 / A1, 3)
    for t in range(4):
        c0 = WQC["l2w"] + 1280 * t
        for pi, (j0, j1, _s) in enumerate(L2PAIRS):
            wq[:, c0 + 256 * pi:c0 + 256 * pi + 128] = l2[t, j0]
            if j1 is not None:
                wq[:, c0 + 256 * pi + 128:c0 + 256 * pi + 256] = l2[t, j1]

    # L3 pairs padded to stride 48 (ldweights dual-fp8 needs 16B-aligned
    # ktile step)
    l3 = _blkdiag(np.asarray(inp["t3_w"], f32), 32.0 * A3 / A2, 4)
    for t in range(4):
        c0 = WQC["l3w"] + 768 * t
        for pi, (j0, j1, _s) in enumerate(L3PAIRS):
            wq[:, c0 + 96 * pi:c0 + 96 * pi + 40] = l3[t, j0]
            wq[:, c0 + 96 * pi + 48:c0 + 96 * pi + 88] = l3[t, j1]

    o["wq"] = wq.astype(FP8)

    # ----- wb (bf16) -----
    wb = np.zeros((128, WB_TOT), f32)

    def srep(aw, taps, kw, nout):
        a = np.asarray(aw, f32)
        out = np.zeros((32, taps * nout), f32)
        for oi in range(taps):
            d, e = divmod(oi, kw)
            out[:, nout * oi:nout * (oi + 1)] = np.repeat(
                a[:, :, d, e].sum(axis=0).reshape(32, 1), nout, 1)
        return out

    wb[:32, WBC["s1w"]:WBC["s1w"] + 288] = srep(inp["a1_w"], 9, 3, 32)
    wb[:32, WBC["s2w"]:WBC["s2w"] + 288] = srep(inp["a2_w"], 9, 3, 32)
    wb[:32, WBC["s3w"]:WBC["s3w"] + 160] = srep(inp["a3_w"], 16, 4, 10)

    # L1 bias field B[t, m, a, b] = sum_taps sum_cc l1[t,tap,cc,m]*bhat(cc,p')
    pc_b = np.asarray(inp["pc_b"], f32)
    bfield = np.zeros((4, 128, 6, 6), np.float64)
    for a in range(6):
        for b in range(6):
            for (d, e) in OFF3:
                pp = 14 * (2 * a + d) + (2 * b + e)
                oi = 3 * d + e
                for t in range(4):
                    cc = 128 * t + np.arange(128)
                    bh = pc_b[32 + (512 * pp + cc) // 196]
                    bfield[t, :, a, b] += l1[t, oi].T @ bh
    t1b_s = A1 * np.asarray(inp["t1_b"], f32).reshape(4, 128)
    for t in range(4):
        for a in range(6):
            for b in range(6):
                wb[:, WBC["b1f"] + 36 * t + 6 * a + b] = \
                    bfield[t, :, a, b] + t1b_s[t]
    o["wb"] = wb.astype(BF)

    # ----- wh (fp16) -----
    wh = np.zeros((128, 168), f32)
    for t in range(4):
        for r in range(128):
            wh[r, 32 * t + (128 * t + r) // 16] = -LAM
        for r in range(40):
            wh[r, 128 + 10 * t + (40 * t + r) // 16] = -LAM
    o["wh"] = wh.astype(F16)

    # ----- wf (f32) -----
    wf = np.zeros((128, 20), f32)
    wf[:, 0] = np.tile(np.asarray(inp["conv1_b"], f32), 4)
    wf[:, 1] = np.tile(pc_b[:32], 4)
    t1b = np.asarray(inp["t1_b"], f32).reshape(4, 128)
    t2b = np.asarray(inp["t2_b"], f32).reshape(4, 128)
    t3b = np.asarray(inp["t3_b"], f32).reshape(4, 40)
    for t in range(4):
        wf[:, 2 + t] = A1 * t1b[t]
        wf[:, 6 + t] = A2 * t2b[t]
        wf[:40, 10 + t] = A3 * t3b[t]
    wf[:32, 14] = (np.asarray(inp["bv1"], f32)[0, 0, 0].sum(1)
                   + C0CONST + 16.0 * np.log(A1))
    wf[:32, 15] = 3.0 * np.asarray(inp["ba1"], f32)[0, 0, 0]
    wf[:32, 16] = (np.asarray(inp["bv2"], f32)[0, 0, 0].sum(1)
                   + C0CONST + 16.0 * np.log(A2))
    wf[:32, 17] = 3.0 * np.asarray(inp["ba2"], f32)[0, 0, 0]
    wf[:10, 18] = (np.asarray(inp["bv3"], f32)[0, 0, 0].sum(1)
                   + C0CONST + 16.0 * np.log(A3))
    wf[:10, 19] = 3.0 * np.asarray(inp["ba3"], f32)[0, 0, 0]
    o["wf"] = wf
    return o


def prep_patches(x_core):
    """im2col for conv1: [128, 1568] fp8; row 32q+(5a+b), col img*196+pos."""
    xp = np.zeros((IMG, 33, 33), np.float32)
    xp[:, 2:30, 2:30] = x_core.reshape(IMG, 28, 28)
    pt = np.zeros((128, 1568), np.float32)
    jj, kk = np.meshgrid(np.arange(14), np.arange(14), indexing="ij")
    for q in range(4):
        for a in range(5):
            for b in range(5):
                for ii in range(8):
                    vals = xp[8 * q + ii, 2 * jj + a, 2 * kk + b]
                    pt[32 * q + 5 * a + b,
                       196 * ii:196 * (ii + 1)] = vals.ravel()
    return pt.astype(FP8)


# ---------------- bass kernel builder ----------------
def build_bass(debug=False, split_waits=True):
    import concourse.bass as bass
    import concourse.tile as tile
    from concourse import mybir
    from contextlib import ExitStack

    f32 = mybir.dt.float32
    bf16 = mybir.dt.bfloat16
    fp16 = mybir.dt.float16
    fp8 = mybir.dt.float8e4
    i16 = mybir.dt.int16
    AF = mybir.ActivationFunctionType
    ALU = mybir.AluOpType
    PM = mybir.MatmulPerfMode

    nc = bass.Bass("TRN2", target_bir_lowering=False, debug=False,
                   num_devices=NCORES)

    din = {}
    din["patches"] = nc.declare_dram_parameter("patches", [128, 1568], fp8,
                                               isOutput=False)
    din["wq"] = nc.declare_dram_parameter("wq", [128, QW_TOT], fp8,
                                          isOutput=False)
    din["wb"] = nc.declare_dram_parameter("wb", [128, WB_TOT], bf16,
                                          isOutput=False)
    din["wh"] = nc.declare_dram_parameter("wh", [128, 168], fp16,
                                          isOutput=False)
    din["wf"] = nc.declare_dram_parameter("wf", [128, 20], f32,
                                          isOutput=False)
    outp = nc.declare_dram_parameter("out", [IMG, 10], f32, isOutput=True)
    dbg = {}
    if debug:
        for nm, shp, dt_ in [("d_h", (128, 1568), fp8),
                             ("d_ppA", (128, 6272), bf16),
                             ("d_acts0", (128, 6272), bf16),
                             ("d_v1", (128, N1), fp8),
                             ("d_sq1", (128, N1), bf16),
                             ("d_ao1", (32, N1), bf16),
                             ("d_v2", (128, N2), fp8),
                             ("d_ao2", (32, N2), bf16),
                             ("d_cs3", (10, 32), f32),
                             ("d_s3", (10, 32), f32)]:
            dbg[nm] = nc.declare_dram_parameter(nm, list(shp), dt_,
                                                isOutput=True)

    ypd = nc.dram_tensor("yp_scr", [IMG * 512 * P0 // 2], bf16)
    yad = nc.dram_tensor("ya_scr", [IMG * 32 * P0 + 128], bf16)

    def dview(handle, off, dims):
        a0 = handle.ap() if hasattr(handle, "ap") else handle
        return bass.AP(tensor=a0.tensor, offset=off,
                       ap=[list(d) for d in dims])

    def sview(tl, p0, np_, off, dims):
        """AP over tile `tl`: partitions [p0, p0+np_), free offset `off`
        (element units of tl's dtype), free dims [[stride, n], ...]."""
        pst = tl.ap[0][0]
        return bass.AP(tensor=tl.tensor, offset=tl.offset + p0 * pst + off,
                       ap=[[pst, np_]] + [list(d) for d in dims])

    with tile.TileContext(nc) as tc, ExitStack() as ctx:
        wpool = ctx.enter_context(tc.tile_pool(name="w", bufs=1))
        dpool = ctx.enter_context(tc.tile_pool(name="d", bufs=1))
        ypool = ctx.enter_context(tc.tile_pool(name="y", bufs=3))
        pmm = ctx.enter_context(tc.tile_pool(name="pmm", bufs=4, space="PSUM"))
        pss = ctx.enter_context(tc.tile_pool(name="pss", bufs=2, space="PSUM"))

        def mmtile():
            ps_mm = pmm.tile([128, 512], f32, tag="mm", name="ps_mm",
                             padded_shape=[128, 512])
            return ps_mm

        def sstile(tag):
            ps_ss = pss.tile([40, 512], f32, tag=tag, name="ps_" + tag,
                             padded_shape=[40, 512])
            return ps_ss

        # ---- weight + patch loads (conv1/pc prerequisites first) ----
        patches = dpool.tile([128, 1568], fp8, tag="patches")
        nc.sync.dma_start(out=patches, in_=din["patches"].ap())
        WQ = wpool.tile([128, QW_TOT], fp8, tag="wq")
        nc.sync.dma_start(out=WQ[:, 0:576], in_=din["wq"].ap()[:, 0:576])
        WF = wpool.tile([128, 20], f32, tag="wf")
        nc.sync.dma_start(out=WF, in_=din["wf"].ap())
        nc.sync.dma_start(out=WQ[:, 576:QW_TOT],
                          in_=din["wq"].ap()[:, 576:QW_TOT])
        WB = wpool.tile([128, WB_TOT], bf16, tag="wb")
        nc.sync.dma_start(out=WB, in_=din["wb"].ap())
        WH = wpool.tile([128, 168], fp16, tag="wh")
        nc.sync.dma_start(out=WH, in_=din["wh"].ap())

        # ---- conv1 ----
        h = dpool.tile([128, 1568], fp8, tag="h")
        for c in range(4):
            ps = mmtile()
            for q in range(4):
                nc.tensor.matmul(
                    out=ps[32 * q:32 * q + 32, 0:392],
                    lhsT=WQ[32 * q:32 * q + 25, WQC["w1"]:WQC["w1"] + 32],
                    rhs=patches[32 * q:32 * q + 25, 392 * c:392 * (c + 1)],
                    start=True, stop=True,
                    tile_position=(32 * q, 32 * q))
            if c % 2 == 0:
                nc.scalar.activation(out=h[:, 392 * c:392 * (c + 1)],
                                     in_=ps[:, 0:392],
                                     func=AF.Relu, bias=WF[:, 0:1],
                                     scale=1.0)
            else:
                nc.vector.tensor_scalar(out=h[:, 392 * c:392 * (c + 1)],
                                        in0=ps[:, 0:392],
                                        scalar1=WF[:, 0:1], scalar2=0.0,
                                        op0=ALU.add, op1=ALU.max)
        if debug:
            nc.sync.dma_start(out=dbg["d_h"].ap(), in_=h)

        # ---- primary caps + staging (per q) ----
        ppA = dpool.tile([128, 6272], bf16, tag="ppA")
        ppB = dpool.tile([128, 6272], bf16, tag="ppB")
        sa = dpool.tile([128, 1568], bf16, tag="sa")
        acts0 = dpool.tile([128, 6272], bf16, tag="acts0")

        def tr_pose(qq, half):
            pp = ppA if half == 0 else ppB
            nc.sync.dma_start(
                out=pp[:, 1568 * qq:1568 * (qq + 1)],
                in_=dview(ypd, qq * 8 * 50176 + 128 * half,
                          [[256, 1568], [1, 128]]),
                transpose=True)

        def tr_acts(qq):
            nc.sync.dma_start(
                out=acts0[:, 1568 * qq:1568 * (qq + 1)],
                in_=dview(yad, qq * 8 * 32 * P0, [[32, 1568], [1, 128]]),
                transpose=True)
        def pose_write(q, mt, ysb):
            wdst = dview(ypd, q * 8 * 50176 + mt * 12544,
                         [[98, 128], [50176, 8], [1, 98]]).bitcast(fp8)
            nc.sync.dma_start(
                out=wdst,
                in_=sview(ysb, 0, 128, 1568 * mt, [[196, 8], [1, 196]]))

        evac_rr = 0
        for q in range(4):
            ysb = ypool.tile([128, 6272], fp8, tag="ysb")
            for mt in range(4):
                for c in range(4):
                    ps = mmtile()
                    nc.tensor.matmul(
                        out=ps[:, 0:392],
                        lhsT=WQ[32 * q:32 * q + 32,
                                WQC["pcw"] + 128 * mt:
                                WQC["pcw"] + 128 * (mt + 1)],
                        rhs=h[32 * q:32 * q + 32, 392 * c:392 * (c + 1)],
                        start=True, stop=True,
                        tile_position=(32 * q, 0))
                    dst = ysb[:, 1568 * mt + 392 * c:
                              1568 * mt + 392 * (c + 1)]
                    if evac_rr % 2 == 0:
                        nc.scalar.activation(out=dst, in_=ps[:, 0:392],
                                             func=AF.Identity, scale=1.0)
                    else:
                        nc.vector.tensor_copy(out=dst, in_=ps[:, 0:392])
                    evac_rr += 1
                pose_write(q, mt, ysb)
                if mt == 1 and q >= 1:
                    tr_pose(q - 1, 0)

        tr_pose(3, 0)
        for qq in range(4):
            tr_pose(qq, 1)
        # acts conv + sigmoid + staging (h still resident)
        for q in range(4):
            for c in range(4):
                ps = mmtile()
                nc.tensor.matmul(
                    out=ps[32 * q:32 * q + 32, 0:392],
                    lhsT=WQ[32 * q:32 * q + 32,
                            WQC["pcw"] + 512:WQC["pcw"] + 544],
                    rhs=h[32 * q:32 * q + 32, 392 * c:392 * (c + 1)],
                    start=True, stop=True,
                    tile_position=(32 * q, 32 * q))
                nc.scalar.activation(out=sa[32 * q:32 * q + 32,
                                            392 * c:392 * (c + 1)],
                                     in_=ps[32 * q:32 * q + 32, 0:392],
                                     func=AF.Sigmoid,
                                     bias=WF[32 * q:32 * q + 32, 1:2],
                                     scale=1.0)
            nc.sync.dma_start(
                out=dview(yad, q * 8 * 32 * P0,
                          [[P0, 32], [32 * P0, 8], [1, P0]]),
                in_=sview(sa, 32 * q, 32, 0, [[196, 8], [1, 196]]))
        for qq in range(4):
            tr_acts(qq)
        if debug:
            nc.sync.dma_start(out=dbg["d_ppA"].ap(), in_=ppA)
            nc.sync.dma_start(out=dbg["d_acts0"].ap(), in_=acts0)

        # ---- L1 votes: DoubleRow over channel pairs ----
        v1 = [dpool.tile([128, N1], fp8, tag=f"v1_{t}", name=f"v1_{t}")
              for t in range(4)]
        sq1 = [dpool.tile([128, N1], bf16, tag=f"sq1_{t}", name=f"sq1_{t}")
               for t in range(4)]
        lpt1 = [dpool.tile([128, N1], fp16, tag=f"lp1_{t}", name=f"lp1_{t}")
                for t in range(4)]
        for t in range(4):
            ppf = (ppA if t < 2 else ppB).bitcast(fp8)
            pb = 64 * (t % 2)
            lc0 = WQC["l1w"] + 2304 * (t // 2)
            for P in range(3):          # a-pairs
                for ah in range(2):
                    a = 2 * P + ah
                    psf = mmtile()
                    ps = psf[:, 0:192]
                    for oi, (d, e) in enumerate(OFF3):
                        rhs = sview(ppf, pb, 64, 56 * a + 28 * d + 2 * e,
                                    [[1, 2], [392, 32], [4, 6]])
                        lhsT = sview(WQ, pb, 64, lc0 + 256 * oi,
                                     [[128, 2], [1, 128]])
                        nc.tensor.matmul(
                            out=ps, lhsT=lhsT, rhs=rhs,
                            start=(oi == 0), stop=(oi == 8),
                            perf_mode=PM.DoubleRow, skip_group_check=True)
                    # evac: v1 = psum + B'(m, a, b) (bias field incl vb1)
                    vsl = sview(v1[t], 0, 128, 12 * P + 6 * ah,
                                [[36, 32], [1, 6]])
                    nc.vector.scalar_tensor_tensor(
                        out=vsl, in0=ps, scalar=1.0,
                        in1=sview(WB, 0, 128,
                                  WBC["b1f"] + 36 * t + 6 * a,
                                  [[0, 32], [1, 6]]),
                        op0=ALU.mult, op1=ALU.add)
                    nc.scalar.activation(
                        out=sview(sq1[t], 0, 128, 12 * P + 6 * ah,
                                  [[36, 32], [1, 6]]),
                        in_=vsl, func=AF.Square, scale=1.0)
            nc.gpsimd.tensor_copy(out=lpt1[t], in_=sq1[t].bitcast(i16))
        if debug:
            nc.sync.dma_start(out=dbg["d_v1"].ap(), in_=v1[0])
            nc.sync.dma_start(out=dbg["d_sq1"].ap(), in_=sq1[0])

        # ---- L2 votes: DoubleRow over tap pairs ----
        v2 = [dpool.tile([128, N2], fp8, tag=f"v2_{t}", name=f"v2_{t}")
              for t in range(4)]
        sq2 = [dpool.tile([128, N2], bf16, tag=f"sq2_{t}", name=f"sq2_{t}")
               for t in range(4)]
        lpt2 = [dpool.tile([128, N2], fp16, tag=f"lp2_{t}", name=f"lp2_{t}")
                for t in range(4)]
        for t in range(4):
            lc0 = WQC["l2w"] + 1280 * t
            for a in range(4):
                psf = mmtile()
                ps = psf[:, 0:128]
                for pi, (j0, j1, kst) in enumerate(L2PAIRS):
                    d0, e0 = divmod(j0, 3)
                    rhs = sview(v1[t], 0, 128, 6 * (a + d0) + e0,
                                [[kst, 2], [36, 32], [1, 4]])
                    lhsT = sview(WQ, 0, 128, lc0 + 256 * pi,
                                 [[128, 2], [1, 128]])
                    nc.tensor.matmul(
                        out=ps, lhsT=lhsT, rhs=rhs,
                        start=(pi == 0), stop=(pi == 4),
                        perf_mode=PM.DoubleRow, skip_group_check=True)
                nc.vector.tensor_scalar(
                    out=sview(v2[t], 0, 128, 4 * a, [[16, 32], [1, 4]]),
                    in0=ps, scalar1=WF[:, 6 + t:7 + t], scalar2=None,
                    op0=ALU.add)
                nc.scalar.activation(
                    out=sview(sq2[t], 0, 128, 4 * a, [[16, 32], [1, 4]]),
                    in_=ps, func=AF.Square, bias=WF[:, 6 + t:7 + t],
                    scale=1.0)
            nc.gpsimd.tensor_copy(out=lpt2[t], in_=sq2[t].bitcast(i16))
        if debug:
            nc.sync.dma_start(out=dbg["d_v2"].ap(), in_=v2[0])

        # ---- L3 ----
        sq3 = [dpool.tile([40, 32], bf16, tag=f"sq3_{t}", name=f"sq3_{t}")
               for t in range(4)]
        lpt3 = [dpool.tile([40, 32], fp16, tag=f"lp3_{t}", name=f"lp3_{t}")
                for t in range(4)]
        for t in range(4):
            lc0 = WQC["l3w"] + 768 * t
            ps = sstile("cs")[0:40, 0:32]
            for pi, (j0, j1, kst) in enumerate(L3PAIRS):
                d0, e0 = divmod(j0, 4)
                rhs = sview(v2[t], 0, 128, 4 * d0 + e0,
                            [[kst, 2], [16, 32]])
                lhsT = sview(WQ, 0, 128, lc0 + 96 * pi, [[48, 2], [1, 40]])
                nc.tensor.matmul(out=ps, lhsT=lhsT, rhs=rhs,
                                 start=(pi == 0), stop=(pi == 7),
                                 perf_mode=PM.DoubleRow,
                                 skip_group_check=True)
            nc.scalar.activation(out=sq3[t], in_=ps, func=AF.Square,
                                 bias=WF[:40, 10 + t:11 + t], scale=1.0)
            nc.gpsimd.tensor_copy(out=lpt3[t], in_=sq3[t].bitcast(i16))
        # ---- L1 routing ----
        ao1 = dpool.tile([32, N1], bf16, tag="ao1")
        for (i0, i1) in L1CH:
            c0, n = 36 * i0, 36 * (i1 - i0)
            pcs = sstile("cs")[0:32, 0:n]
            for t in range(4):
                nc.tensor.matmul(out=pcs, lhsT=WH[:, 32 * t:32 * t + 32],
                                 rhs=lpt1[t][:, c0:c0 + n],
                                 start=(t == 0), stop=(t == 3))
            psr = sstile("sr")[0:32, 0:n]
            for oi, (d, e) in enumerate(OFF3):
                rhs = sview(acts0, 0, 32, 196 * i0 + 14 * d + e,
                            [[196, i1 - i0], [28, 6], [2, 6]])
                nc.tensor.matmul(out=psr,
                                 lhsT=WB[:32, WBC["s1w"] + 32 * oi:
                                         WBC["s1w"] + 32 * (oi + 1)],
                                 rhs=rhs, start=(oi == 0), stop=(oi == 8))
            csb = dpool.tile([32, 512], f32, tag="csb", name=f"csb{i0}")
            nc.vector.tensor_scalar(out=csb[:, :n], in0=pcs,
                                    scalar1=WF[:32, 14:15], scalar2=None,
                                    op0=ALU.add)
            tm = dpool.tile([32, 512], f32, tag="tm", name=f"tm{i0}")
            nc.vector.tensor_mul(tm[:, :n], csb[:, :n], psr)
            nc.scalar.activation(out=ao1[:, c0:c0 + n], in_=tm[:, :n],
                                 func=AF.Sigmoid, bias=WF[:32, 15:16],
                                 scale=-3.0)
        if debug:
            nc.sync.dma_start(out=dbg["d_ao1"].ap(), in_=ao1)

        # ---- L2 routing ----
        ao2 = dpool.tile([32, N2], bf16, tag="ao2")
        pcs2 = sstile("cs")[0:32, :]
        for t in range(4):
            nc.tensor.matmul(out=pcs2, lhsT=WH[:, 32 * t:32 * t + 32],
                             rhs=lpt2[t], start=(t == 0), stop=(t == 3))
        psr2 = sstile("sr")[0:32, :]
        for oi, (d, e) in enumerate(OFF3):
            rhs = sview(ao1, 0, 32, 6 * d + e,
                        [[36, 32], [6, 4], [1, 4]])
            nc.tensor.matmul(out=psr2,
                             lhsT=WB[:32, WBC["s2w"] + 32 * oi:
                                     WBC["s2w"] + 32 * (oi + 1)],
                             rhs=rhs, start=(oi == 0), stop=(oi == 8))
        csb2 = dpool.tile([32, 512], f32, tag="csb", name="csb2")
        nc.vector.tensor_scalar(out=csb2, in0=pcs2, scalar1=WF[:32, 16:17],
                                scalar2=None, op0=ALU.add)
        tm2 = dpool.tile([32, 512], f32, tag="tm", name="tm2")
        nc.vector.tensor_mul(tm2, csb2, psr2)
        nc.scalar.activation(out=ao2, in_=tm2, func=AF.Sigmoid,
                             bias=WF[:32, 17:18], scale=-3.0)
        if debug:
            nc.sync.dma_start(out=dbg["d_ao2"].ap(), in_=ao2)

        pcs3 = sstile("cs")[0:10, 0:32]
        for t in range(4):
            nc.tensor.matmul(out=pcs3,
                             lhsT=WH[:40, 128 + 10 * t:128 + 10 * (t + 1)],
                             rhs=lpt3[t], start=(t == 0), stop=(t == 3))
        psr3 = sstile("sr")[0:10, 0:32]
        for oi in range(16):
            d, e = divmod(oi, 4)
            rhs = sview(ao2, 0, 32, 4 * d + e, [[16, 32]])
            nc.tensor.matmul(out=psr3,
                             lhsT=WB[:32, WBC["s3w"] + 10 * oi:
                                     WBC["s3w"] + 10 * (oi + 1)],
                             rhs=rhs, start=(oi == 0), stop=(oi == 15))
        csb3 = dpool.tile([10, 32], f32, tag="csb3")
        nc.vector.tensor_scalar(out=csb3, in0=pcs3, scalar1=WF[:10, 18:19],
                                scalar2=None, op0=ALU.add)
        tm3 = dpool.tile([10, 32], f32, tag="tm3")
        nc.vector.tensor_mul(tm3, csb3, psr3)
        ao3 = dpool.tile([10, 32], f32, tag="ao3")
        nc.scalar.activation(out=ao3, in_=tm3, func=AF.Sigmoid,
                             bias=WF[:10, 19:20], scale=-3.0)
        if debug:
            nc.sync.dma_start(out=dbg["d_cs3"].ap(), in_=csb3)
            s3f = dpool.tile([10, 32], f32, tag="s3f")
            nc.vector.tensor_copy(out=s3f, in_=psr3)
            nc.sync.dma_start(out=dbg["d_s3"].ap(), in_=s3f)

        # ---- output: (10,32) -> DRAM (32,10) ----
        nc.sync.dma_start(out=dview(outp, 0, [[1, 10], [10, 32]]), in_=ao3)

    if split_waits:
        split_sync_waits(nc, max_waits=1)
    return nc


def split_sync_waits(nc, max_waits=1):
    """Walrus in this environment encodes at most `max_waits` semaphore
    waits per instruction; hoist extras onto preceding same-engine NoOps."""
    from concourse import mybir
    n_split = 0
    for blk in nc.m.functions[0].blocks:
        insts = list(blk.instructions)
        out = []
        for ins in insts:
            si = ins.sync_info
            if si is not None and si.on_wait and len(si.on_wait) > max_waits:
                waits = list(si.on_wait)
                extras, keep = waits[:-max_waits], waits[-max_waits:]
                for w in extras:
                    nop = mybir.InstNoOp(
                        name=nc.get_next_instruction_name(), ins=[], outs=[])
                    nop.engine = ins.engine
                    nop.sync_info = mybir.SyncInfo(on_wait=[w], on_update=[])
                    out.append(nop)
                ins.sync_info = mybir.SyncInfo(
                    on_wait=keep, on_update=list(si.on_update or []))
                n_split += 1
            out.append(ins)
        if len(out) != len(insts):
            blk.instructions = out
    return n_split


def make_in_maps(inputs):
    w = prep_weights(inputs)
    x = np.asarray(inputs["x"], np.float32).reshape(BATCH, 784)
    in_maps = []
    for c in range(NCORES):
        m = {nm: w[nm] for nm in ("wq", "wb", "wh", "wf")}
        m["patches"] = prep_patches(x[c * IMG:(c + 1) * IMG])
        in_maps.append(m)
    return in_maps


# ---------------- entry point ----------------
_CACHE = {}


def kernel(**inputs):
    from concourse.bass_utils import run_bass_kernel_spmd

    if "nc" not in _CACHE:
        _CACHE["nc"] = build_bass(debug=False)
    nc = _CACHE["nc"]
    res = run_bass_kernel_spmd(nc, make_in_maps(inputs), list(range(NCORES)))
    return np.concatenate([np.asarray(r["out"], np.float32)
                           for r in res.results], axis=0)
